# revision 15
# baseline (speedup 1.0000x reference)
"""Trainium2 Bass kernel for nn_MetaMultiLinear (bf16, transpose-free, v6).

Math (per head h, sample b):
    w[b, k]   = sum_c cond[b, c] * CW[k, c] + cb[k]        k = o*17 + i  (544)
    out[b, o] = sum_i x1[b, i] * w[b, o*17+i]              x1 = [input, 1] (17)

Sharding: head h -> NeuronCore h (8 heads, 8 cores), full B=32768 per core.

All data bf16 (fp32 PSUM accumulation; harness gate 2e-2, lands ~4e-3).
This PE runs at a fixed 1.2 GHz (cold formula (219+N)/1.2 always), DVE
0.96 GHz, ACT ~0.63 GHz effective — so the kernel LOAD-BALANCES the
per-sample multiply+reduce across PE, DVE and ACT with two block routes:

  P (PE reduce):  DVE muls w1 straight from PSUM fp32 (1x) into (i,o)
     tmp; per-tile PE identity-matmul streams tmp (N=512 contiguous)
     accumulating the 16 i-passes onto po via the broadcast out-AP;
     ACT copies po -> outs.
  B (ACT+DVE):   W-MM streams the (o,i)-ordered weight copy cwk_oi, so
     ACT's pair-fused PSUM->SBUF bf16 copy yields (o,i) w1s; DVE muls
     at the fast 2x-ish bf16 SBUF rate (~0.4 ns/elem); a 4-pair-fused
     binary add tree on DVE (amortizes the ~85ns/op overhead) plus a
     fused final add with the ACT-copied po finishes the reduce.

Host pre-packs everything in on-chip layout (partition-contiguous DMAs),
including cond1^T, so there is no on-chip transpose:
  ct  [128, NP, 128]: pair p, col j: rows 0:33 = [cond|1]^T of sample
      256p+j (tile 2p), rows 64:97 = same for tile 2p+1.
  xs  [128, NP, 32]:  xs[p, pr, t*16+i] = input[256pr + 128t + p, i]
  cwk/cwk_oi [128, 512]: rows g:g+33 (g=0,64) = CW[(i<16,o),c] c-major,
      i-major / o-major column order, bias row at g+32.
  cwo [128, 32]:      i=16 (x1=1) slice; opens the out accumulation.
  out [128, NP, 64]   bf16 (t*32+o), host converts back to fp32 [B, 32].

Fully unrolled (a For_i back-edge is a barrier that blocks DMA prefetch).
Two strip-concurrent matmuls must never drain to the same PSUM bank
(device wedge), hence per-tile po banks.
"""

import sys

import numpy as np

if "/opt/trn_rl_repo" not in sys.path:
    sys.path.insert(0, "/opt/trn_rl_repo")

N_HEADS, IN_F, COND_IN, OUT_F = 8, 16, 32, 32
B = 32768
INP1 = IN_F + 1  # 17
KW = OUT_F * IN_F  # 512 (i<16 part)
C1 = COND_IN + 1  # 33
P = 128
NT = B // P  # 256 tiles
NP = NT // 2  # 128 pairs
BLK = 4  # pairs per route block (tree fusion unit)
DMA_CHUNK = 8  # pairs per DMA

# route per 4-pair block, cycled: P = PE reduce, B = ACT-copy + DVE tree
BLOCK_ROUTES = "PPB"

_cached_nc = None


def _build_nc(n_pairs=NP):
    import concourse.bass as bass
    import concourse.mybir as mybir
    import concourse.tile as tile
    from concourse import bacc
    from contextlib import ExitStack

    f32 = mybir.dt.float32
    bf16 = mybir.dt.bfloat16
    nc = bacc.Bacc()

    ct_t = nc.dram_tensor("ct", [P, n_pairs, P], bf16, kind="ExternalInput")
    xs_t = nc.dram_tensor("xs", [P, n_pairs, 2 * IN_F], bf16, kind="ExternalInput")
    cwk_t = nc.dram_tensor("cwk", [P, KW], bf16, kind="ExternalInput")
    cwk_oi_t = nc.dram_tensor("cwk_oi", [P, KW], bf16, kind="ExternalInput")
    cwo_t = nc.dram_tensor("cwo", [P, OUT_F], bf16, kind="ExternalInput")
    ident_t = nc.dram_tensor("ident", [P, P], bf16, kind="ExternalInput")
    out_t = nc.dram_tensor("out", [P, n_pairs, 2 * OUT_F], bf16, kind="ExternalOutput")

    n_chunks = n_pairs // DMA_CHUNK

    with tile.TileContext(nc) as tc, ExitStack() as ctx:
        consts = ctx.enter_context(tc.tile_pool(name="consts", bufs=1))
        pin = ctx.enter_context(tc.tile_pool(name="pin", bufs=4))
        ptmp = ctx.enter_context(tc.tile_pool(name="ptmp", bufs=3))
        pw1s = ctx.enter_context(tc.tile_pool(name="pw1s", bufs=3))
        ptree = ctx.enter_context(tc.tile_pool(name="ptree", bufs=2))
        pposb = ctx.enter_context(tc.tile_pool(name="pposb", bufs=3))
        pouts = ctx.enter_context(tc.tile_pool(name="pouts", bufs=3))
        pps_w = ctx.enter_context(tc.tile_pool(name="pps_w", bufs=2, space="PSUM"))
        pps_o = ctx.enter_context(tc.tile_pool(name="pps_o", bufs=2, space="PSUM"))

        cwk = consts.tile([P, KW], bf16)
        nc.sync.dma_start(out=cwk, in_=cwk_t[:])
        cwk_oi = consts.tile([P, KW], bf16)
        nc.sync.dma_start(out=cwk_oi, in_=cwk_oi_t[:])
        cwo = consts.tile([P, OUT_F], bf16)
        nc.sync.dma_start(out=cwo, in_=cwo_t[:])
        idn = consts.tile([P, P], bf16)
        nc.sync.dma_start(out=idn, in_=ident_t[:])

        outs_by_chunk = {}

        def hyper(pr, route, ct_c, j):
            """W + po matmuls for pair pr; returns (w1, po)."""
            cts = ct_c[:, j, :]
            w1 = pps_w.tile([P, 2, KW], f32)
            po = pps_o.tile([P, 2, 512], f32)
            wmat = cwk_oi if route == "B" else cwk
            for t in (0, 1):
                g = 64 * t
                ctsl = cts[g : g + C1, :]
                nc.tensor.matmul(
                    w1[:, t, :],
                    ctsl,
                    wmat[g : g + C1, :],
                    start=True,
                    stop=True,
                    tile_position=(g, 0),
                )
                nc.tensor.matmul(
                    po[:, t, 0:OUT_F],
                    ctsl,
                    cwo[g : g + C1, :],
                    start=True,
                    stop=(route == "B"),
                    skip_group_check=True,
                    tile_position=(g, 0),
                )
            return w1, po

        def emit_block_P(b0, ct_c, xs_c, outs_c, j0):
            for k in range(BLK):
                pr, j = b0 + k, j0 + k
                w1, po = hyper(pr, "P", ct_c, j)
                xst = xs_c[:, j, :].rearrange("p (t i) -> p t i", t=2)
                tmp = ptmp.tile([P, 2, IN_F, OUT_F], bf16, tag="tmpP")
                w1v = w1[:].rearrange("p t (i o) -> p t i o", o=OUT_F)
                xv = xst[:].unsqueeze(3).broadcast_to([P, 2, IN_F, OUT_F])
                nc.vector.tensor_mul(tmp[:], w1v, xv)
                for t in (0, 1):
                    pv = (
                        po[:, t, 0:OUT_F]
                        .unsqueeze(1)
                        .broadcast_to([P, IN_F, OUT_F])
                    )
                    nc.tensor.matmul(
                        pv,
                        idn[:],
                        tmp[:, t, :, :],
                        start=False,
                        stop=True,
                        skip_group_check=True,
                    )
                ov = outs_c[:, j, :].rearrange("p (t o) -> p t o", t=2)
                nc.scalar.copy(out=ov, in_=po[:, :, 0:OUT_F])

        def emit_block_B(b0, ct_c, xs_c, outs_c, j0):
            # r8 holds l1 results of all BLK pairs: [P, 2*BLK tiles, 32, 8]
            r8 = ptree.tile([P, 2 * BLK, OUT_F, 8], bf16, tag="r8")
            poS = pposb.tile([P, 2 * BLK, OUT_F], bf16)
            for k in range(BLK):
                pr, j = b0 + k, j0 + k
                w1, po = hyper(pr, "B", ct_c, j)
                xst = xs_c[:, j, :].rearrange("p (t i) -> p t i", t=2)
                # ACT evacuates the pair's w1 (o,i) to SBUF bf16; per-tile
                # ops so the first copy starts as soon as tile 0's W-MM lands
                w1s = pw1s.tile([P, 2, KW], bf16)
                nc.scalar.copy(out=w1s[:, 0, :], in_=w1[:, 0, :])
                nc.scalar.copy(out=w1s[:, 1, :], in_=w1[:, 1, :])
                # fast bf16 SBUF mul: innermost i step-1 on both operands
                tmp = ptmp.tile([P, 2, OUT_F, IN_F], bf16, tag="tmpB")
                w1v = w1s[:].rearrange("p t (o i) -> p t o i", i=IN_F)
                xv = xst[:].unsqueeze(2).broadcast_to([P, 2, OUT_F, IN_F])
                nc.vector.tensor_mul(tmp[:], w1v, xv)
                # l1 for this pair into the shared r8
                nc.vector.tensor_add(
                    r8[:, 2 * k : 2 * k + 2, :, :],
                    tmp[:, :, :, 0:8],
                    tmp[:, :, :, 8:16],
                )
                nc.scalar.copy(out=poS[:, 2 * k : 2 * k + 2, :], in_=po[:, :, 0:OUT_F])
            # block-fused l2..l4 + final add
            r4 = ptree.tile([P, 2 * BLK, OUT_F, 4], bf16, tag="r4")
            nc.vector.tensor_add(r4[:], r8[:, :, :, 0:4], r8[:, :, :, 4:8])
            r2 = ptree.tile([P, 2 * BLK, OUT_F, 2], bf16, tag="r2")
            nc.vector.tensor_add(r2[:], r4[:, :, :, 0:2], r4[:, :, :, 2:4])
            r1 = ptree.tile([P, 2 * BLK, OUT_F], bf16, tag="r1")
            nc.vector.tensor_add(r1[:], r2[:, :, :, 0], r2[:, :, :, 1])
            ov4 = outs_c[:, j0 : j0 + BLK, :].rearrange(
                "p j (t o) -> p (j t) o", t=2
            )
            nc.vector.tensor_add(ov4, r1[:], poS[:])

        for ch in range(n_chunks):
            p0 = ch * DMA_CHUNK
            ct_c = pin.tile([P, DMA_CHUNK, P], bf16, tag="ct_c")
            xs_c = pin.tile([P, DMA_CHUNK, 2 * IN_F], bf16, tag="xs_c")
            if ch == 0:
                # micro-DMAs so the first pairs' data lands ASAP
                for mp in range(0, DMA_CHUNK, 2):
                    nc.sync.dma_start(
                        out=ct_c[:, mp : mp + 2, :],
                        in_=ct_t[:, p0 + mp : p0 + mp + 2, :],
                    )
                    nc.sync.dma_start(
                        out=xs_c[:, mp : mp + 2, :],
                        in_=xs_t[:, p0 + mp : p0 + mp + 2, :],
                    )
            else:
                nc.sync.dma_start(out=ct_c[:], in_=ct_t[:, p0 : p0 + DMA_CHUNK, :])
                nc.sync.dma_start(out=xs_c[:], in_=xs_t[:, p0 : p0 + DMA_CHUNK, :])
            outs_c = pouts.tile([P, DMA_CHUNK, 2 * OUT_F], bf16)

            for bi in range(DMA_CHUNK // BLK):
                b0 = p0 + bi * BLK
                blk_idx = b0 // BLK
                route = BLOCK_ROUTES[blk_idx % len(BLOCK_ROUTES)]
                if route == "P":
                    emit_block_P(b0, ct_c, xs_c, outs_c, bi * BLK)
                else:
                    emit_block_B(b0, ct_c, xs_c, outs_c, bi * BLK)

            nc.sync.dma_start(
                out=out_t[:, p0 : p0 + DMA_CHUNK, :],
                in_=outs_c[:],
            )

    nc.compile()
    return nc


def _get_nc():
    global _cached_nc
    if _cached_nc is None:
        _cached_nc = _build_nc()
    return _cached_nc


def _make_in_maps(input, cond, cond_weight, cond_bias):
    import ml_dtypes

    bf = ml_dtypes.bfloat16
    ident = np.eye(P, dtype=bf)
    in_maps = []
    n_heads, b_total = input.shape[0], input.shape[1]
    npair = b_total // 256
    for h in range(n_heads):
        cond1 = np.concatenate(
            [cond[h], np.ones((b_total, 1), np.float32)], axis=1
        )  # [B, 33]
        c4 = cond1.reshape(npair, 2, P, C1).astype(bf)  # [p, t, j, c]
        ct = np.zeros((P, npair, P), bf)
        ct[0:C1] = c4[:, 0].transpose(2, 0, 1)
        ct[64 : 64 + C1] = c4[:, 1].transpose(2, 0, 1)
        xs = (
            input[h]
            .reshape(npair, 2, P, IN_F)
            .transpose(2, 0, 1, 3)
            .reshape(P, npair, 2 * IN_F)
            .astype(bf)
        )
        cw3 = cond_weight[h].reshape(OUT_F, INP1, COND_IN)  # (o, i, c)
        cb2 = cond_bias[h].reshape(OUT_F, INP1)  # (o, i)

        def pack_w(order):
            # order "io": cols k' = i*32+o; order "oi": cols k' = o*16+i
            if order == "io":
                w1 = cw3[:, :IN_F, :].transpose(2, 1, 0).reshape(COND_IN, KW)
                br = cb2[:, :IN_F].T.reshape(KW)
            else:
                w1 = cw3[:, :IN_F, :].transpose(2, 0, 1).reshape(COND_IN, KW)
                br = cb2[:, :IN_F].reshape(KW)
            m = np.zeros((P, KW), bf)
            m[0:COND_IN] = w1.astype(bf)
            m[COND_IN] = br.astype(bf)
            m[64 : 64 + COND_IN] = w1.astype(bf)
            m[64 + COND_IN] = br.astype(bf)
            return m

        cwk = pack_w("io")
        cwk_oi = pack_w("oi")
        cwo = np.zeros((P, OUT_F), bf)
        cwo[0:COND_IN] = cw3[:, IN_F, :].T.astype(bf)
        cwo[COND_IN] = cb2[:, IN_F].astype(bf)
        cwo[64 : 64 + COND_IN] = cw3[:, IN_F, :].T.astype(bf)
        cwo[64 + COND_IN] = cb2[:, IN_F].astype(bf)
        in_maps.append(
            {"ct": ct, "xs": xs, "cwk": cwk, "cwk_oi": cwk_oi, "cwo": cwo, "ident": ident}
        )
    return in_maps


def _run(in_maps, **kwargs):
    from concourse import bass_utils

    nc = _get_nc()
    return bass_utils.run_bass_kernel_spmd(
        nc, in_maps, core_ids=list(range(N_HEADS)), **kwargs
    )


def _unpack_out(res):
    # out [128, NP, 64] bf16 -> [B, 32] fp32 per head
    outs = []
    for r in res.results:
        o = np.asarray(r["out"], dtype=np.float32)  # [P, NP, 64]
        o = o.reshape(P, -1, 2, OUT_F).transpose(1, 2, 0, 3).reshape(-1, OUT_F)
        outs.append(o)
    return np.stack(outs, axis=0)


def kernel(input, cond, cond_weight, cond_bias):
    input = np.asarray(input, np.float32)
    cond = np.asarray(cond, np.float32)
    cond_weight = np.asarray(cond_weight, np.float32)
    cond_bias = np.asarray(cond_bias, np.float32)
    in_maps = _make_in_maps(input, cond, cond_weight, cond_bias)
    res = _run(in_maps)
    return _unpack_out(res)


# revision 16
# speedup vs baseline: 1.0346x; 1.0346x over previous
"""Trainium2 Bass kernel for nn_MetaMultiLinear (bf16, transpose-free, v6).

Math (per head h, sample b):
    w[b, k]   = sum_c cond[b, c] * CW[k, c] + cb[k]        k = o*17 + i  (544)
    out[b, o] = sum_i x1[b, i] * w[b, o*17+i]              x1 = [input, 1] (17)

Sharding: head h -> NeuronCore h (8 heads, 8 cores), full B=32768 per core.

All data bf16 (fp32 PSUM accumulation; harness gate 2e-2, lands ~4e-3).
This PE runs at a fixed 1.2 GHz (cold formula (219+N)/1.2 always), DVE
0.96 GHz, ACT ~0.63 GHz effective — so the kernel LOAD-BALANCES the
per-sample multiply+reduce across PE, DVE and ACT with two block routes:

  P (PE reduce):  DVE muls w1 straight from PSUM fp32 (1x) into (i,o)
     tmp; per-tile PE identity-matmul streams tmp (N=512 contiguous)
     accumulating the 16 i-passes onto po via the broadcast out-AP;
     ACT copies po -> outs.
  B (ACT+DVE):   W-MM streams the (o,i)-ordered weight copy cwk_oi, so
     ACT's pair-fused PSUM->SBUF bf16 copy yields (o,i) w1s; DVE muls
     at the fast 2x-ish bf16 SBUF rate (~0.4 ns/elem); a 4-pair-fused
     binary add tree on DVE (amortizes the ~85ns/op overhead) plus a
     fused final add with the ACT-copied po finishes the reduce.

Host pre-packs everything in on-chip layout (partition-contiguous DMAs),
including cond1^T, so there is no on-chip transpose:
  ct  [128, NP, 128]: pair p, col j: rows 0:33 = [cond|1]^T of sample
      256p+j (tile 2p), rows 64:97 = same for tile 2p+1.
  xs  [128, NP, 32]:  xs[p, pr, t*16+i] = input[256pr + 128t + p, i]
  cwk/cwk_oi [128, 512]: rows g:g+33 (g=0,64) = CW[(i<16,o),c] c-major,
      i-major / o-major column order, bias row at g+32.
  cwo [128, 32]:      i=16 (x1=1) slice; opens the out accumulation.
  out [128, NP, 64]   bf16 (t*32+o), host converts back to fp32 [B, 32].

Fully unrolled (a For_i back-edge is a barrier that blocks DMA prefetch).
Two strip-concurrent matmuls must never drain to the same PSUM bank
(device wedge), hence per-tile po banks.
"""

import sys

import numpy as np

if "/opt/trn_rl_repo" not in sys.path:
    sys.path.insert(0, "/opt/trn_rl_repo")

N_HEADS, IN_F, COND_IN, OUT_F = 8, 16, 32, 32
B = 32768
INP1 = IN_F + 1  # 17
KW = OUT_F * IN_F  # 512 (i<16 part)
C1 = COND_IN + 1  # 33
P = 128
NT = B // P  # 256 tiles
NP = NT // 2  # 128 pairs
BLK = 4  # pairs per route block (tree fusion unit)
DMA_CHUNK = 8  # pairs per DMA

# route per 4-pair block, cycled: P = PE reduce, B = ACT-copy + DVE tree
BLOCK_ROUTES = "PPB"

_cached_nc = None


def _build_nc(n_pairs=NP):
    import concourse.bass as bass
    import concourse.mybir as mybir
    import concourse.tile as tile
    from concourse import bacc
    from contextlib import ExitStack

    f32 = mybir.dt.float32
    bf16 = mybir.dt.bfloat16
    nc = bacc.Bacc()

    ct_t = nc.dram_tensor("ct", [P, n_pairs, P], bf16, kind="ExternalInput")
    xs_t = nc.dram_tensor("xs", [P, n_pairs, 2 * IN_F], bf16, kind="ExternalInput")
    cwk_t = nc.dram_tensor("cwk", [P, KW], bf16, kind="ExternalInput")
    cwk_oi_t = nc.dram_tensor("cwk_oi", [P, KW], bf16, kind="ExternalInput")
    cwo_t = nc.dram_tensor("cwo", [P, OUT_F], bf16, kind="ExternalInput")
    ident_t = nc.dram_tensor("ident", [P, P], bf16, kind="ExternalInput")
    out_t = nc.dram_tensor("out", [P, n_pairs, 2 * OUT_F], bf16, kind="ExternalOutput")

    n_chunks = n_pairs // DMA_CHUNK

    with tile.TileContext(nc) as tc, ExitStack() as ctx:
        consts = ctx.enter_context(tc.tile_pool(name="consts", bufs=1))
        pin = ctx.enter_context(tc.tile_pool(name="pin", bufs=4))
        ptmp = ctx.enter_context(tc.tile_pool(name="ptmp", bufs=3))
        pw1s = ctx.enter_context(tc.tile_pool(name="pw1s", bufs=3))
        ptree = ctx.enter_context(tc.tile_pool(name="ptree", bufs=2))
        pposb = ctx.enter_context(tc.tile_pool(name="pposb", bufs=3))
        pouts = ctx.enter_context(tc.tile_pool(name="pouts", bufs=3))
        pps_w = ctx.enter_context(tc.tile_pool(name="pps_w", bufs=2, space="PSUM"))
        pps_o = ctx.enter_context(tc.tile_pool(name="pps_o", bufs=2, space="PSUM"))

        cwk = consts.tile([P, KW], bf16)
        nc.sync.dma_start(out=cwk, in_=cwk_t[:])
        cwk_oi = consts.tile([P, KW], bf16)
        nc.sync.dma_start(out=cwk_oi, in_=cwk_oi_t[:])
        cwo = consts.tile([P, OUT_F], bf16)
        nc.sync.dma_start(out=cwo, in_=cwo_t[:])
        idn = consts.tile([P, P], bf16)
        nc.sync.dma_start(out=idn, in_=ident_t[:])

        outs_by_chunk = {}

        def hyper(pr, route, ct_c, j):
            """W + po matmuls for pair pr; returns (w1, po)."""
            cts = ct_c[:, j, :]
            w1 = pps_w.tile([P, 2, KW], f32)
            po = pps_o.tile([P, 2, 512], f32)
            wmat = cwk_oi if route == "B" else cwk
            for t in (0, 1):
                g = 64 * t
                ctsl = cts[g : g + C1, :]
                nc.tensor.matmul(
                    w1[:, t, :],
                    ctsl,
                    wmat[g : g + C1, :],
                    start=True,
                    stop=True,
                    tile_position=(g, 0),
                )
                nc.tensor.matmul(
                    po[:, t, 0:OUT_F],
                    ctsl,
                    cwo[g : g + C1, :],
                    start=True,
                    stop=(route == "B"),
                    skip_group_check=True,
                    tile_position=(g, 0),
                )
            return w1, po

        def emit_block_P(b0, ct_c, xs_c, outs_c, j0):
            for k in range(BLK):
                pr, j = b0 + k, j0 + k
                w1, po = hyper(pr, "P", ct_c, j)
                xst = xs_c[:, j, :].rearrange("p (t i) -> p t i", t=2)
                tmp = ptmp.tile([P, 2, IN_F, OUT_F], bf16, tag="tmpP")
                w1v = w1[:].rearrange("p t (i o) -> p t i o", o=OUT_F)
                xv = xst[:].unsqueeze(3).broadcast_to([P, 2, IN_F, OUT_F])
                nc.vector.tensor_mul(tmp[:], w1v, xv)
                for t in (0, 1):
                    pv = (
                        po[:, t, 0:OUT_F]
                        .unsqueeze(1)
                        .broadcast_to([P, IN_F, OUT_F])
                    )
                    nc.tensor.matmul(
                        pv,
                        idn[:],
                        tmp[:, t, :, :],
                        start=False,
                        stop=True,
                        skip_group_check=True,
                    )
                ov = outs_c[:, j, :].rearrange("p (t o) -> p t o", t=2)
                nc.scalar.copy(out=ov, in_=po[:, :, 0:OUT_F])

        def emit_block_B(b0, ct_c, xs_c, outs_c, j0):
            # r8 holds l1 results of all BLK pairs: [P, 2*BLK tiles, 32, 8]
            r8 = ptree.tile([P, 2 * BLK, OUT_F, 8], bf16, tag="r8")
            poS = pposb.tile([P, 2 * BLK, OUT_F], bf16)
            for k in range(BLK):
                pr, j = b0 + k, j0 + k
                w1, po = hyper(pr, "B", ct_c, j)
                xst = xs_c[:, j, :].rearrange("p (t i) -> p t i", t=2)
                # ACT evacuates the pair's w1 (o,i) to SBUF bf16
                w1s = pw1s.tile([P, 2, KW], bf16)
                nc.scalar.copy(out=w1s[:], in_=w1[:])
                # fast bf16 SBUF mul: innermost i step-1 on both operands
                tmp = ptmp.tile([P, 2, OUT_F, IN_F], bf16, tag="tmpB")
                w1v = w1s[:].rearrange("p t (o i) -> p t o i", i=IN_F)
                xv = xst[:].unsqueeze(2).broadcast_to([P, 2, OUT_F, IN_F])
                nc.vector.tensor_mul(tmp[:], w1v, xv)
                # l1 for this pair into the shared r8
                nc.vector.tensor_add(
                    r8[:, 2 * k : 2 * k + 2, :, :],
                    tmp[:, :, :, 0:8],
                    tmp[:, :, :, 8:16],
                )
                nc.scalar.copy(out=poS[:, 2 * k : 2 * k + 2, :], in_=po[:, :, 0:OUT_F])
            # block-fused l2..l4 + final add
            r4 = ptree.tile([P, 2 * BLK, OUT_F, 4], bf16, tag="r4")
            nc.vector.tensor_add(r4[:], r8[:, :, :, 0:4], r8[:, :, :, 4:8])
            r2 = ptree.tile([P, 2 * BLK, OUT_F, 2], bf16, tag="r2")
            nc.vector.tensor_add(r2[:], r4[:, :, :, 0:2], r4[:, :, :, 2:4])
            r1 = ptree.tile([P, 2 * BLK, OUT_F], bf16, tag="r1")
            nc.vector.tensor_add(r1[:], r2[:, :, :, 0], r2[:, :, :, 1])
            ov4 = outs_c[:, j0 : j0 + BLK, :].rearrange(
                "p j (t o) -> p (j t) o", t=2
            )
            nc.vector.tensor_add(ov4, r1[:], poS[:])

        for ch in range(n_chunks):
            p0 = ch * DMA_CHUNK
            ct_c = pin.tile([P, DMA_CHUNK, P], bf16, tag="ct_c")
            xs_c = pin.tile([P, DMA_CHUNK, 2 * IN_F], bf16, tag="xs_c")
            if ch == 0:
                # micro-DMAs so the first pairs' data lands ASAP
                for mp in range(0, DMA_CHUNK, 2):
                    nc.sync.dma_start(
                        out=ct_c[:, mp : mp + 2, :],
                        in_=ct_t[:, p0 + mp : p0 + mp + 2, :],
                    )
                    nc.sync.dma_start(
                        out=xs_c[:, mp : mp + 2, :],
                        in_=xs_t[:, p0 + mp : p0 + mp + 2, :],
                    )
            else:
                nc.sync.dma_start(out=ct_c[:], in_=ct_t[:, p0 : p0 + DMA_CHUNK, :])
                nc.sync.dma_start(out=xs_c[:], in_=xs_t[:, p0 : p0 + DMA_CHUNK, :])
            outs_c = pouts.tile([P, DMA_CHUNK, 2 * OUT_F], bf16)

            for bi in range(DMA_CHUNK // BLK):
                b0 = p0 + bi * BLK
                blk_idx = b0 // BLK
                route = BLOCK_ROUTES[blk_idx % len(BLOCK_ROUTES)]
                if route == "P":
                    emit_block_P(b0, ct_c, xs_c, outs_c, bi * BLK)
                else:
                    emit_block_B(b0, ct_c, xs_c, outs_c, bi * BLK)

            nc.sync.dma_start(
                out=out_t[:, p0 : p0 + DMA_CHUNK, :],
                in_=outs_c[:],
            )

    nc.compile()
    return nc


def _get_nc():
    global _cached_nc
    if _cached_nc is None:
        _cached_nc = _build_nc()
    return _cached_nc


def _make_in_maps(input, cond, cond_weight, cond_bias):
    import ml_dtypes

    bf = ml_dtypes.bfloat16
    ident = np.eye(P, dtype=bf)
    in_maps = []
    n_heads, b_total = input.shape[0], input.shape[1]
    npair = b_total // 256
    for h in range(n_heads):
        cond1 = np.concatenate(
            [cond[h], np.ones((b_total, 1), np.float32)], axis=1
        )  # [B, 33]
        c4 = cond1.reshape(npair, 2, P, C1).astype(bf)  # [p, t, j, c]
        ct = np.zeros((P, npair, P), bf)
        ct[0:C1] = c4[:, 0].transpose(2, 0, 1)
        ct[64 : 64 + C1] = c4[:, 1].transpose(2, 0, 1)
        xs = (
            input[h]
            .reshape(npair, 2, P, IN_F)
            .transpose(2, 0, 1, 3)
            .reshape(P, npair, 2 * IN_F)
            .astype(bf)
        )
        cw3 = cond_weight[h].reshape(OUT_F, INP1, COND_IN)  # (o, i, c)
        cb2 = cond_bias[h].reshape(OUT_F, INP1)  # (o, i)

        def pack_w(order):
            # order "io": cols k' = i*32+o; order "oi": cols k' = o*16+i
            if order == "io":
                w1 = cw3[:, :IN_F, :].transpose(2, 1, 0).reshape(COND_IN, KW)
                br = cb2[:, :IN_F].T.reshape(KW)
            else:
                w1 = cw3[:, :IN_F, :].transpose(2, 0, 1).reshape(COND_IN, KW)
                br = cb2[:, :IN_F].reshape(KW)
            m = np.zeros((P, KW), bf)
            m[0:COND_IN] = w1.astype(bf)
            m[COND_IN] = br.astype(bf)
            m[64 : 64 + COND_IN] = w1.astype(bf)
            m[64 + COND_IN] = br.astype(bf)
            return m

        cwk = pack_w("io")
        cwk_oi = pack_w("oi")
        cwo = np.zeros((P, OUT_F), bf)
        cwo[0:COND_IN] = cw3[:, IN_F, :].T.astype(bf)
        cwo[COND_IN] = cb2[:, IN_F].astype(bf)
        cwo[64 : 64 + COND_IN] = cw3[:, IN_F, :].T.astype(bf)
        cwo[64 + COND_IN] = cb2[:, IN_F].astype(bf)
        in_maps.append(
            {"ct": ct, "xs": xs, "cwk": cwk, "cwk_oi": cwk_oi, "cwo": cwo, "ident": ident}
        )
    return in_maps


def _run(in_maps, **kwargs):
    from concourse import bass_utils

    nc = _get_nc()
    return bass_utils.run_bass_kernel_spmd(
        nc, in_maps, core_ids=list(range(N_HEADS)), **kwargs
    )


def _unpack_out(res):
    # out [128, NP, 64] bf16 -> [B, 32] fp32 per head
    outs = []
    for r in res.results:
        o = np.asarray(r["out"], dtype=np.float32)  # [P, NP, 64]
        o = o.reshape(P, -1, 2, OUT_F).transpose(1, 2, 0, 3).reshape(-1, OUT_F)
        outs.append(o)
    return np.stack(outs, axis=0)


def kernel(input, cond, cond_weight, cond_bias):
    input = np.asarray(input, np.float32)
    cond = np.asarray(cond, np.float32)
    cond_weight = np.asarray(cond_weight, np.float32)
    cond_bias = np.asarray(cond_bias, np.float32)
    in_maps = _make_in_maps(input, cond, cond_weight, cond_bias)
    res = _run(in_maps)
    return _unpack_out(res)
